# revision 3
# baseline (speedup 1.0000x reference)
"""Trainium2 Bass kernel for nn_Conv_27693949125154.

Each 128-dim vector is a 16x8 image; valid 3x3 conv with the fixed kernel
[[1,0,1],[0,1,0],[1,0,1]] then relu -> 84 outputs (14x6).

The conv kernel decomposes as outer([1,0,1],[1,0,1]) + center tap:
    h(i,j) = x(i,j) + x(i,j+2)            (horizontal, 16x6)
    out(i,j) = relu(h(i,j) + h(i+2,j) + x(i+1,j+1))   (14x6)

Layout: letters (B*W rows) on SBUF partitions, the 128 pixels of each
letter along the free dim. All 5 stencil taps become free-dim strided
slices, so the whole conv is 3 DVE tensor-adds + 1 ACT relu per tile.
Pure data parallel over 8 NeuronCores (batch sharding, no comm).
"""

import numpy as np

import concourse.bass as bass
import concourse.mybir as mybir
from concourse import tile
from concourse.bass_utils import run_bass_kernel_spmd

# Full problem: x (16384, 14, 128) f32 -> out (16384, 14, 84) f32
B, W, L = 16384, 14, 128
OUT = 84
N_CORES = 8
ROWS = B * W                 # 229376 letters total
ROWS_PER_CORE = ROWS // N_CORES  # 28672
P = 128                      # SBUF partitions

F32 = mybir.dt.float32


def split_multi_waits(nc, max_waits=1):
    """walrus CoreV3 codegen rejects instructions with several sync-wait
    conditions; hoist extras onto NOPs inserted just before, same engine."""
    for f in nc.m.functions:
        for blk in f.blocks:
            new = []
            for inst in blk.instructions:
                si = inst.sync_info
                if si is not None and si.on_wait and len(si.on_wait) > max_waits:
                    waits = list(si.on_wait)
                    head, tail = waits[:-max_waits], waits[-max_waits:]
                    for k, w in enumerate(head):
                        new.append(
                            mybir.InstNoOp(
                                name=f"{inst.name}-wsplit{k}",
                                engine=inst.engine,
                                ins=[],
                                outs=[],
                                sync_info=mybir.SyncInfo(on_wait=[w], on_update=[]),
                            )
                        )
                    inst.sync_info = mybir.SyncInfo(
                        on_wait=tail, on_update=list(si.on_update)
                    )
                new.append(inst)
            blk.instructions = new


def build_program(rows=ROWS_PER_CORE, t_letters=32, split_waits=True):
    """Per-core program: x [rows,128] f32 -> y [rows,84] f32.

    Tiles of 128*t_letters rows; each partition holds t_letters letters
    contiguously in the free dim.
    """
    T = t_letters
    assert rows % (P * T) == 0
    n_tiles = rows // (P * T)

    nc = bass.Bass(
        "TRN2", target_bir_lowering=False, debug=False, num_devices=N_CORES
    )
    x = nc.dram_tensor("x", [rows, L], F32, kind="ExternalInput")
    y = nc.dram_tensor("y", [rows, OUT], F32, kind="ExternalOutput")

    # partition p of tile n holds letters [ (n*128+p)*T, (n*128+p)*T + T )
    xv = x.ap().rearrange("(n p t) m -> n p (t m)", p=P, t=T)
    yv = y.ap().rearrange("(n p t) m -> n p (t m)", p=P, t=T)

    with tile.TileContext(nc) as tc:
        with (
            tc.tile_pool(name="io", bufs=3) as io_pool,
            tc.tile_pool(name="work", bufs=2) as work,
        ):
            for n in range(n_tiles):
                xt = io_pool.tile([P, T * L], F32, tag="x")
                nc.sync.dma_start(out=xt[:], in_=xv[n])

                # h(i,j) = x(i,j) + x(i,j+2): rows are (T*16) x 8, keep cols 0:6
                r = work.tile([P, T * 96], F32, tag="r")
                x3 = xt.rearrange("p (row c) -> p row c", c=8)     # [P, T*16, 8]
                r3 = r.rearrange("p (row c) -> p row c", c=6)      # [P, T*16, 6]
                nc.vector.tensor_tensor(
                    r3[:], x3[:, :, 0:6], x3[:, :, 2:8], mybir.AluOpType.add
                )

                # s = h(rows 0..13) + center taps x(1..14, 1..6)
                s = work.tile([P, T * 84], F32, tag="s")
                r4 = r.rearrange("p (t i j) -> p t i j", i=16, j=6)  # [P,T,16,6]
                x4 = xt.rearrange("p (t i j) -> p t i j", i=16, j=8)  # [P,T,16,8]
                s4 = s.rearrange("p (t i j) -> p t i j", i=14, j=6)
                nc.vector.tensor_tensor(
                    s4[:], r4[:, :, 0:14, :], x4[:, :, 1:15, 1:7], mybir.AluOpType.add
                )

                # u = s + h(rows 2..15)
                u = work.tile([P, T * 84], F32, tag="u")
                u4 = u.rearrange("p (t i j) -> p t i j", i=14, j=6)
                nc.vector.tensor_tensor(
                    u4[:], s4[:], r4[:, :, 2:16, :], mybir.AluOpType.add
                )

                # relu on the scalar engine (keeps DVE free)
                ot = io_pool.tile([P, T * OUT], F32, tag="o")
                nc.scalar.activation(
                    ot[:], u[:], mybir.ActivationFunctionType.Relu
                )
                nc.sync.dma_start(out=yv[n], in_=ot[:])

    if split_waits:
        split_multi_waits(nc)
    return nc


_nc_cache = {}


def _get_program():
    if "nc" not in _nc_cache:
        _nc_cache["nc"] = build_program()
    return _nc_cache["nc"]


def kernel(x):
    x = np.ascontiguousarray(np.asarray(x, dtype=np.float32))
    assert x.shape == (B, W, L), x.shape

    nc = _get_program()
    shards = x.reshape(N_CORES, ROWS_PER_CORE, L)
    in_maps = [{"x": shards[i]} for i in range(N_CORES)]
    res = run_bass_kernel_spmd(nc, in_maps, core_ids=list(range(N_CORES)))
    out = np.concatenate(
        [res.results[i]["y"].reshape(-1, W, OUT) for i in range(N_CORES)], axis=0
    )
    return out


# revision 5
# speedup vs baseline: 1.2872x; 1.2872x over previous
"""Trainium2 Bass kernel for nn_Conv_27693949125154.

Each 128-dim vector is a 16x8 image; valid 3x3 conv with the fixed kernel
[[1,0,1],[0,1,0],[1,0,1]] then relu -> 84 outputs (14x6).

The conv kernel decomposes as outer([1,0,1],[1,0,1]) + center tap:
    h(i,j) = x(i,j) + x(i,j+2)            (horizontal, 16x6)
    out(i,j) = relu(h(i,j) + h(i+2,j) + x(i+1,j+1))   (14x6)

Layout: letters (B*W rows) on SBUF partitions, the 128 pixels of each
letter along the free dim. All 5 stencil taps become free-dim strided
slices, so the whole conv is 3 DVE tensor-adds + 1 ACT relu per tile.
Pure data parallel over 8 NeuronCores (batch sharding, no comm).
"""

import numpy as np

import concourse.bass as bass
import concourse.mybir as mybir
from concourse import tile
from concourse.bass_utils import run_bass_kernel_spmd

# Full problem: x (16384, 14, 128) f32 -> out (16384, 14, 84) f32
B, W, L = 16384, 14, 128
OUT = 84
N_CORES = 8
ROWS = B * W                 # 229376 letters total
ROWS_PER_CORE = ROWS // N_CORES  # 28672
P = 128                      # SBUF partitions

F32 = mybir.dt.float32


def split_multi_waits(nc, max_waits=1):
    """walrus CoreV3 codegen rejects instructions with several sync-wait
    conditions; hoist extras onto NOPs inserted just before, same engine."""
    for f in nc.m.functions:
        for blk in f.blocks:
            new = []
            for inst in blk.instructions:
                si = inst.sync_info
                if si is not None and si.on_wait and len(si.on_wait) > max_waits:
                    waits = list(si.on_wait)
                    head, tail = waits[:-max_waits], waits[-max_waits:]
                    for k, w in enumerate(head):
                        new.append(
                            mybir.InstNoOp(
                                name=f"{inst.name}-wsplit{k}",
                                engine=inst.engine,
                                ins=[],
                                outs=[],
                                sync_info=mybir.SyncInfo(on_wait=[w], on_update=[]),
                            )
                        )
                    inst.sync_info = mybir.SyncInfo(
                        on_wait=tail, on_update=list(si.on_update)
                    )
                new.append(inst)
            blk.instructions = new


def build_program(rows=ROWS_PER_CORE, t_letters=32, split_waits=True,
                  op1_engine="gpsimd"):
    """Per-core program: x [rows,128] f32 -> y [rows,84] f32.

    Tiles of 128*t_letters rows; each partition holds t_letters letters
    contiguously in the free dim.
    """
    T = t_letters
    assert rows % (P * T) == 0
    n_tiles = rows // (P * T)

    nc = bass.Bass(
        "TRN2", target_bir_lowering=False, debug=False, num_devices=N_CORES
    )
    x = nc.dram_tensor("x", [rows, L], F32, kind="ExternalInput")
    y = nc.dram_tensor("y", [rows, OUT], F32, kind="ExternalOutput")

    # partition p of tile n holds letters [ (n*128+p)*T, (n*128+p)*T + T )
    xv = x.ap().rearrange("(n p t) m -> n p (t m)", p=P, t=T)
    yv = y.ap().rearrange("(n p t) m -> n p (t m)", p=P, t=T)

    with tile.TileContext(nc) as tc:
        with (
            tc.tile_pool(name="io", bufs=3) as io_pool,
            tc.tile_pool(name="work", bufs=2) as work,
        ):
            for n in range(n_tiles):
                xt = io_pool.tile([P, T * L], F32, tag="x")
                nc.sync.dma_start(out=xt[:], in_=xv[n])

                # h(i,j) = x(i,j) + x(i,j+2): rows are (T*16) x 8, keep cols 0:6
                r = work.tile([P, T * 96], F32, tag="r")
                x3 = xt.rearrange("p (row c) -> p row c", c=8)     # [P, T*16, 8]
                r3 = r.rearrange("p (row c) -> p row c", c=6)      # [P, T*16, 6]
                op1 = nc.gpsimd if op1_engine == "gpsimd" else nc.vector
                op1.tensor_tensor(
                    r3[:], x3[:, :, 0:6], x3[:, :, 2:8], mybir.AluOpType.add
                )

                # s = h(rows 0..13) + center taps x(1..14, 1..6)
                s = work.tile([P, T * 84], F32, tag="s")
                r4 = r.rearrange("p (t i j) -> p t i j", i=16, j=6)  # [P,T,16,6]
                x4 = xt.rearrange("p (t i j) -> p t i j", i=16, j=8)  # [P,T,16,8]
                s4 = s.rearrange("p (t i j) -> p t i j", i=14, j=6)
                nc.vector.tensor_tensor(
                    s4[:], r4[:, :, 0:14, :], x4[:, :, 1:15, 1:7], mybir.AluOpType.add
                )

                # u = s + h(rows 2..15)
                u = work.tile([P, T * 84], F32, tag="u")
                u4 = u.rearrange("p (t i j) -> p t i j", i=14, j=6)
                nc.vector.tensor_tensor(
                    u4[:], s4[:], r4[:, :, 2:16, :], mybir.AluOpType.add
                )

                # relu on the scalar engine (keeps DVE free)
                ot = io_pool.tile([P, T * OUT], F32, tag="o")
                nc.scalar.activation(
                    ot[:], u[:], mybir.ActivationFunctionType.Relu
                )
                nc.sync.dma_start(out=yv[n], in_=ot[:])

    if split_waits:
        split_multi_waits(nc)
    return nc


_nc_cache = {}


def _get_program():
    if "nc" not in _nc_cache:
        _nc_cache["nc"] = build_program()
    return _nc_cache["nc"]


def kernel(x):
    x = np.ascontiguousarray(np.asarray(x, dtype=np.float32))
    assert x.shape == (B, W, L), x.shape

    nc = _get_program()
    shards = x.reshape(N_CORES, ROWS_PER_CORE, L)
    in_maps = [{"x": shards[i]} for i in range(N_CORES)]
    res = run_bass_kernel_spmd(nc, in_maps, core_ids=list(range(N_CORES)))
    out = np.concatenate(
        [res.results[i]["y"].reshape(-1, W, OUT) for i in range(N_CORES)], axis=0
    )
    return out
